# revision 1
# baseline (speedup 1.0000x reference)
"""Trainium2 Bass kernel for nn_ChunkwiseMLSTM (B=2, S=8192, D=512, INNER=1024, NH=8).

kernel(**inputs) -> np.ndarray [2, 8192, 512] f32.

Three SPMD launches on 8 NeuronCores:
  A: token-sharded projections (up-proj, causal conv, silu, q/k/v, gate pre-acts)
  B: head-sharded chunkwise mLSTM (L=128 chunks; f32 state, bf16 matmuls)
  C: token-sharded output gating + down-projection
Host between launches: gate-derived scan scalars / decay matrices (O(B*NH*S) work),
weight pre-transposition, and resharding.
"""
import os
os.environ.setdefault("JAX_COMPILATION_CACHE_DIR",
                      os.path.expanduser("~/.cache/jax_bass_cache"))
os.environ.setdefault("JAX_PERSISTENT_CACHE_MIN_ENTRY_SIZE_BYTES", "0")
os.environ.setdefault("JAX_PERSISTENT_CACHE_MIN_COMPILE_TIME_SECS", "0")

import sys
if '/opt/trn_rl_repo' not in sys.path:
    sys.path.insert(0, '/opt/trn_rl_repo')

import numpy as np
import ml_dtypes

import concourse.bass as bass
import concourse.tile as tile
from concourse import mybir, bacc

F32 = mybir.dt.float32
BF16 = mybir.dt.bfloat16
AF = mybir.ActivationFunctionType
OP = mybir.AluOpType

B, S, D = 2, 8192, 512
INNER, NH, KCONV = 1024, 8, 4
DH = 128
EPS = 1e-6
LC = 128           # chunk length used on device (math is chunk-size invariant)
NCH = S // LC      # 64
QK_SCALE = DH ** -0.5
TOK = S // 4       # tokens per core in phases A/C = 2048
TH = TOK + (KCONV - 1)   # 2051 with conv halo
NUNIT = 2          # (b,h) units per core in phase B


def _bf(x):
    return np.ascontiguousarray(np.asarray(x, np.float32).astype(ml_dtypes.bfloat16))


def new_nc():
    return bacc.Bacc(None, target_bir_lowering=False, debug=False)


# ---------------------------------------------------------------- phase A ----
def build_phase_a():
    nc = new_nc()
    xt = nc.dram_tensor("xt", [D, TH], BF16, kind="ExternalInput")            # x[b].T slice (halo)
    wupT = nc.dram_tensor("wupT", [D, 2 * INNER], BF16, kind="ExternalInput")  # W_up.T
    wqkvT = nc.dram_tensor("wqkvT", [INNER, 3 * INNER], BF16, kind="ExternalInput")
    wgT = nc.dram_tensor("wgT", [INNER, 2 * NH], BF16, kind="ExternalInput")   # [Wig.T | Wfg.T]
    # host-built diagonal conv weight tiles: diag[p, ft, t, col] = conv_w[ft*128+p, t] * (col==p)
    diag_i = nc.dram_tensor("diag_i", [DH, (INNER // DH) * KCONV * DH], BF16, kind="ExternalInput")
    convb = nc.dram_tensor("convb", [DH, INNER // DH], F32, kind="ExternalInput")

    q_o = nc.dram_tensor("q_o", [INNER, TOK], BF16, kind="ExternalOutput")
    k_o = nc.dram_tensor("k_o", [INNER, TOK], BF16, kind="ExternalOutput")
    v_o = nc.dram_tensor("v_o", [INNER, TOK], BF16, kind="ExternalOutput")
    xm_o = nc.dram_tensor("xm_o", [INNER, TOK], BF16, kind="ExternalOutput")
    xog_o = nc.dram_tensor("xog_o", [INNER, TOK], BF16, kind="ExternalOutput")
    gates_o = nc.dram_tensor("gates_o", [2 * NH, TOK], F32, kind="ExternalOutput")

    KT_UP = D // DH          # 4 k-tiles for up-proj
    MT_UP = 2 * INNER // DH  # 16 m-tiles
    FT = INNER // DH         # 8 feature tiles of the mlstm half
    KT_IN = INNER // DH      # 8 k-tiles over INNER
    MT_QKV = 3 * INNER // DH # 24
    # token n-tiles over TH (matmul N <= 512)
    N_SL = [(i * 512, min(512, TH - i * 512)) for i in range((TH + 511) // 512)]
    N_SL_OUT = [(i * 512, 512) for i in range(TOK // 512)]

    with tile.TileContext(nc) as tc, \
         tc.tile_pool(name="const", bufs=1) as const, \
         tc.tile_pool(name="big", bufs=1) as big, \
         tc.tile_pool(name="wpool", bufs=3) as wpool, \
         tc.tile_pool(name="ev", bufs=4) as ev, \
         tc.tile_pool(name="ps", bufs=4, space="PSUM") as ps:
        if True:
            # --- load x and W_up
            xt_sb = big.tile([DH, KT_UP, TH], BF16)
            nc.sync.dma_start(out=xt_sb, in_=xt[:].rearrange("(kt p) t -> p kt t", p=DH))
            wup_sb = big.tile([DH, KT_UP, 2 * INNER], BF16)
            nc.sync.dma_start(out=wup_sb, in_=wupT[:].rearrange("(kt p) m -> p kt m", p=DH))

            # --- up-projection: x_pre (mlstm half, bf16, kept) and x_og (exported)
            xpre_sb = big.tile([DH, FT, TH], BF16)
            xog_sb = big.tile([DH, FT, TOK], BF16)
            for m in range(MT_UP):
                mlstm_half = m < FT
                for (n0, nn) in (N_SL if mlstm_half else N_SL_OUT):
                    pt = ps.tile([DH, 512], F32)
                    off = 0 if mlstm_half else KCONV - 1
                    for kt in range(KT_UP):
                        nc.tensor.matmul(
                            pt[:, :nn],
                            wup_sb[:, kt, m * DH:(m + 1) * DH],
                            xt_sb[:, kt, off + n0: off + n0 + nn],
                            start=(kt == 0), stop=(kt == KT_UP - 1))
                    if mlstm_half:
                        nc.scalar.copy(xpre_sb[:, m, n0:n0 + nn], pt[:, :nn])
                    else:
                        nc.vector.tensor_copy(xog_sb[:, m - FT, n0:n0 + nn], pt[:, :nn])
            nc.sync.dma_start(out=xog_o[:].rearrange("(ft p) t -> p ft t", p=DH), in_=xog_sb)

            # --- causal depthwise conv as 4 diag matmuls + SiLU
            convb_sb = const.tile([DH, FT], F32)
            nc.sync.dma_start(out=convb_sb, in_=convb[:])
            diag = const.tile([DH, FT, KCONV, DH], BF16)
            nc.sync.dma_start(
                out=diag, in_=diag_i[:].rearrange("p (ft t c) -> p ft t c", ft=FT, t=KCONV))

            xm_sb = big.tile([DH, FT, TOK], BF16)
            for ft in range(FT):
                for (n0, nn) in N_SL_OUT:
                    pt = ps.tile([DH, 512], F32)
                    for t in range(KCONV):
                        nc.tensor.matmul(
                            pt[:, :nn],
                            diag[:, ft, t, :],
                            xpre_sb[:, ft, n0 + t: n0 + t + nn],
                            start=(t == 0), stop=(t == KCONV - 1))
                    # silu(y+b) = (y+b) * sigmoid(y+b)   (Silu LUT unavailable in CoreSim)
                    sg_t = ev.tile([DH, 512], BF16, tag="sg")
                    nc.scalar.activation(
                        sg_t[:, :nn], pt[:, :nn], AF.Sigmoid,
                        bias=convb_sb[:, ft:ft + 1], scale=1.0)
                    nc.vector.scalar_tensor_tensor(
                        xm_sb[:, ft, n0:n0 + nn], pt[:, :nn], convb_sb[:, ft:ft + 1],
                        sg_t[:, :nn], OP.add, OP.mult)
            nc.sync.dma_start(out=xm_o[:].rearrange("(ft p) t -> p ft t", p=DH), in_=xm_sb)

            # --- q/k/v projections (streamed weights) + gates
            qkv_outs = [q_o, k_o, v_o]
            for m in range(MT_QKV):
                w_sb = wpool.tile([DH, KT_IN, DH], BF16)
                nc.sync.dma_start(
                    out=w_sb,
                    in_=wqkvT[:, m * DH:(m + 1) * DH].rearrange("(kt p) m -> p kt m", p=DH))
                out_t = qkv_outs[m // FT]
                mf = m % FT
                for (n0, nn) in N_SL_OUT:
                    pt = ps.tile([DH, 512], F32)
                    for kt in range(KT_IN):
                        nc.tensor.matmul(
                            pt[:, :nn], w_sb[:, kt, :], xm_sb[:, kt, n0:n0 + nn],
                            start=(kt == 0), stop=(kt == KT_IN - 1))
                    ev_t = ev.tile([DH, 512], BF16)
                    if m % 2 == 0:
                        nc.scalar.copy(ev_t[:, :nn], pt[:, :nn])
                    else:
                        nc.vector.tensor_copy(ev_t[:, :nn], pt[:, :nn])
                    nc.sync.dma_start(
                        out=out_t[mf * DH:(mf + 1) * DH, n0:n0 + nn], in_=ev_t[:, :nn])

            # gates: [16, TOK]
            wg_sb = const.tile([DH, KT_IN, 2 * NH], BF16)
            nc.sync.dma_start(out=wg_sb, in_=wgT[:].rearrange("(kt p) m -> p kt m", p=DH))
            for (n0, nn) in N_SL_OUT:
                pt = ps.tile([2 * NH, 512], F32)
                for kt in range(KT_IN):
                    nc.tensor.matmul(
                        pt[:, :nn], wg_sb[:, kt, :], xm_sb[:, kt, n0:n0 + nn],
                        start=(kt == 0), stop=(kt == KT_IN - 1))
                gv = ev.tile([2 * NH, 512], F32)
                nc.vector.tensor_copy(gv[:, :nn], pt[:, :nn])
                nc.sync.dma_start(out=gates_o[:, n0:n0 + nn], in_=gv[:, :nn])
    nc.compile()
    return nc


# ---------------------------------------------------------------- phase B ----
def build_phase_b(interleave=True, cast_engine="gpsimd", ablate=()):
    ablate = set(ablate)
    nc = new_nc()
    NW = 132  # padded width for [C|n] and [v|1]
    ins = {}
    outs = {}
    for u in range(NUNIT):
        ins[f"qT{u}"] = nc.dram_tensor(f"qT{u}", [DH, S], BF16, kind="ExternalInput")
        ins[f"kT{u}"] = nc.dram_tensor(f"kT{u}", [DH, S], BF16, kind="ExternalInput")
        ins[f"kesc{u}"] = nc.dram_tensor(f"kesc{u}", [S, DH], BF16, kind="ExternalInput")
        ins[f"vone{u}"] = nc.dram_tensor(f"vone{u}", [S, NW], BF16, kind="ExternalInput")
        ins[f"DpT{u}"] = nc.dram_tensor(f"DpT{u}", [S, DH], BF16, kind="ExternalInput")
        ins[f"dec{u}"] = nc.dram_tensor(f"dec{u}", [DH, NCH], F32, kind="ExternalInput")
        ins[f"e2{u}"] = nc.dram_tensor(f"e2{u}", [DH, NCH], F32, kind="ExternalInput")
        ins[f"e3{u}"] = nc.dram_tensor(f"e3{u}", [DH, NCH], F32, kind="ExternalInput")
        outs[f"h{u}"] = nc.dram_tensor(f"h{u}", [S, DH], F32, kind="ExternalOutput")

    with tile.TileContext(nc) as tc, \
         tc.tile_pool(name="big", bufs=1) as big, \
         tc.tile_pool(name="work", bufs=4) as work, \
         tc.tile_pool(name="hpool", bufs=4) as hpool, \
         tc.tile_pool(name="state", bufs=1) as state, \
         tc.tile_pool(name="ps_s", bufs=2, space="PSUM") as ps_s, \
         tc.tile_pool(name="ps_num", bufs=3, space="PSUM") as ps_num, \
         tc.tile_pool(name="ps_u", bufs=2, space="PSUM") as ps_u:
        if True:
            T = {}
            for u in range(NUNIT):
                T[u] = dict(
                    qT=big.tile([DH, S], BF16, tag=f"qT{u}", name=f"qT{u}"),
                    kT=big.tile([DH, S], BF16, tag=f"kT{u}", name=f"kT{u}"),
                    kesc=big.tile([DH, NCH, DH], BF16, tag=f"kesc{u}", name=f"kesc{u}"),
                    vone=big.tile([DH, NCH, NW], BF16, tag=f"vone{u}", name=f"vone{u}"),
                    DpT=big.tile([DH, NCH, DH], BF16, tag=f"DpT{u}", name=f"DpT{u}"),
                    dec=big.tile([DH, NCH], F32, tag=f"dec{u}", name=f"dec{u}"),
                    e2=big.tile([DH, NCH], F32, tag=f"e2{u}", name=f"e2{u}"),
                    e3=big.tile([DH, NCH], F32, tag=f"e3{u}", name=f"e3{u}"),
                    Cf=state.tile([DH, NW], F32, tag=f"Cf{u}", name=f"Cf{u}"),
                    Cb=state.tile([DH, NW], BF16, tag=f"Cb{u}", name=f"Cb{u}"),
                )
                t = T[u]
                nc.sync.dma_start(out=t['qT'], in_=ins[f"qT{u}"][:])
                nc.sync.dma_start(out=t['kT'], in_=ins[f"kT{u}"][:])
                nc.sync.dma_start(out=t['kesc'], in_=ins[f"kesc{u}"][:].rearrange("(c p) e -> p c e", p=DH))
                nc.sync.dma_start(out=t['vone'], in_=ins[f"vone{u}"][:].rearrange("(c p) e -> p c e", p=DH))
                nc.sync.dma_start(out=t['DpT'], in_=ins[f"DpT{u}"][:].rearrange("(c p) e -> p c e", p=DH))
                nc.sync.dma_start(out=t['dec'], in_=ins[f"dec{u}"][:])
                nc.sync.dma_start(out=t['e2'], in_=ins[f"e2{u}"][:])
                nc.sync.dma_start(out=t['e3'], in_=ins[f"e3{u}"][:])
                nc.vector.memset(t['Cf'][:], 0.0)
                nc.vector.memset(t['Cb'][:], 0.0)

            def chunk_body(u, c):
                t = T[u]
                csl = slice(c * LC, (c + 1) * LC)
                # mm1: S_T[j,l] = k_c @ q_c^T
                s_ps = ps_s.tile([DH, DH], F32, tag="s_ps")
                if 'mm1' not in ablate:
                    nc.tensor.matmul(s_ps[:], t['kT'][:, csl], t['qT'][:, csl], start=True, stop=True)
                # Sp = S_T * Dp^T   (masked, scaled)  -> bf16
                sp = work.tile([DH, DH], BF16, tag="sp")
                if 'sp' not in ablate:
                    nc.vector.tensor_tensor(sp[:], s_ps[:], t['DpT'][:, c, :], OP.mult)
                # num = q_c^T-mm @ [C|n]  +  Sp^T @ [v|1]
                num = ps_num.tile([DH, NW], F32, tag="num")
                if 'num' not in ablate:
                    nc.tensor.matmul(num[:], t['qT'][:, csl], t['Cb'][:], start=True, stop=False)
                    nc.tensor.matmul(num[:], sp[:], t['vone'][:, c, :], start=False, stop=True)
                # den = abs_max(num[:,128], e2) + e3 ; rden = 1/den
                den = work.tile([DH, 1], F32, tag="den")
                absd = work.tile([DH, 1], F32, tag="absd")
                rden = work.tile([DH, 1], F32, tag="rden")
                if 'den' not in ablate:
                    # den = max(|den_raw|, e2) + e3   (abs_max not walrus-legal)
                    nc.scalar.activation(absd[:], num[:, DH:DH + 1], AF.Abs)
                    nc.vector.scalar_tensor_tensor(
                        den[:], absd[:], t['e2'][:, c:c + 1], t['e3'][:, c:c + 1],
                        OP.max, OP.add)
                    nc.vector.reciprocal(rden[:], den[:])
                # h_c = num[:, :128] * rden   (ACT copy with per-partition scale)
                h_sb = hpool.tile([DH, DH], F32, tag="h_sb")
                if 'h' not in ablate:
                    nc.scalar.activation(h_sb[:], num[:, :DH], AF.Copy, bias=0.0, scale=rden[:])
                    nc.sync.dma_start(out=outs[f"h{u}"][csl, :], in_=h_sb[:])
                # mm2: U = kesc_c^T @ [v|1]
                u_ps = ps_u.tile([DH, NW], F32, tag="u_ps")
                if 'mm2' not in ablate:
                    nc.tensor.matmul(u_ps[:], t['kesc'][:, c, :], t['vone'][:, c, :], start=True, stop=True)
                # C = C*dec + U ;  Cb = bf16(C)
                if 'stt' not in ablate:
                    nc.vector.scalar_tensor_tensor(
                        t['Cf'][:], t['Cf'][:], t['dec'][:, c:c + 1], u_ps[:], OP.mult, OP.add)
                if 'cast' not in ablate:
                    if cast_engine == "gpsimd":
                        nc.gpsimd.tensor_copy(t['Cb'][:], t['Cf'][:])
                    else:
                        nc.scalar.copy(t['Cb'][:], t['Cf'][:])

            if interleave:
                for c in range(NCH):
                    for u in range(NUNIT):
                        chunk_body(u, c)
            else:
                for u in range(NUNIT):
                    for c in range(NCH):
                        chunk_body(u, c)
    nc.compile()
    return nc


# ---------------------------------------------------------------- phase C ----
def build_phase_c():
    nc = new_nc()
    h_i = nc.dram_tensor("h_i", [INNER, TOK], BF16, kind="ExternalInput")
    xm_i = nc.dram_tensor("xm_i", [INNER, TOK], BF16, kind="ExternalInput")
    xog_i = nc.dram_tensor("xog_i", [INNER, TOK], BF16, kind="ExternalInput")
    skip_i = nc.dram_tensor("skip_i", [DH, INNER // DH], F32, kind="ExternalInput")
    wdT = nc.dram_tensor("wdT", [INNER, D], BF16, kind="ExternalInput")
    out_o = nc.dram_tensor("out_o", [D, TOK], F32, kind="ExternalOutput")

    FT = INNER // DH   # 8
    MT = D // DH       # 4
    N_SL = [(i * 512, 512) for i in range(TOK // 512)]
    with tile.TileContext(nc) as tc, \
         tc.tile_pool(name="big", bufs=1) as big, \
         tc.tile_pool(name="ev", bufs=4) as ev, \
         tc.tile_pool(name="ps", bufs=4, space="PSUM") as ps:
        if True:
            h_sb = big.tile([DH, FT, TOK], BF16)
            xm_sb = big.tile([DH, FT, TOK], BF16)
            xog_sb = big.tile([DH, FT, TOK], BF16)
            skip_sb = big.tile([DH, FT], F32)
            wd_sb = big.tile([DH, FT, D], BF16)
            nc.sync.dma_start(out=h_sb, in_=h_i[:].rearrange("(ft p) t -> p ft t", p=DH))
            nc.sync.dma_start(out=xm_sb, in_=xm_i[:].rearrange("(ft p) t -> p ft t", p=DH))
            nc.sync.dma_start(out=xog_sb, in_=xog_i[:].rearrange("(ft p) t -> p ft t", p=DH))
            nc.sync.dma_start(out=skip_sb, in_=skip_i[:])
            nc.sync.dma_start(out=wd_sb, in_=wdT[:].rearrange("(ft p) m -> p ft m", p=DH))

            hg_sb = big.tile([DH, FT, TOK], BF16)
            for ft in range(FT):
                g = ev.tile([DH, TOK], BF16, tag="g")
                nc.scalar.activation(g[:], xog_sb[:, ft, :], AF.Sigmoid)
                g2 = ev.tile([DH, TOK], BF16, tag="g2")
                nc.vector.tensor_tensor(g2[:], xog_sb[:, ft, :], g[:], OP.mult)
                hs = ev.tile([DH, TOK], BF16, tag="hs")
                nc.vector.scalar_tensor_tensor(
                    hs[:], xm_sb[:, ft, :], skip_sb[:, ft:ft + 1], h_sb[:, ft, :],
                    OP.mult, OP.add)
                nc.vector.tensor_tensor(hg_sb[:, ft, :], hs[:], g2[:], OP.mult)

            for m in range(MT):
                for (n0, nn) in N_SL:
                    pt = ps.tile([DH, 512], F32)
                    for kt in range(FT):
                        nc.tensor.matmul(
                            pt[:, :nn], wd_sb[:, kt, m * DH:(m + 1) * DH],
                            hg_sb[:, kt, n0:n0 + nn],
                            start=(kt == 0), stop=(kt == FT - 1))
                    ot = ev.tile([DH, 512], F32, tag="ot")
                    nc.vector.tensor_copy(ot[:, :nn], pt[:, :nn])
                    nc.sync.dma_start(out=out_o[m * DH:(m + 1) * DH, n0:n0 + nn], in_=ot[:, :nn])
    nc.compile()
    return nc


# ------------------------------------------------------------- host glue ----
def host_gate_math(i_pre, f_pre):
    """i_pre, f_pre: [B, NH, S] f32.  Returns dict of f32 arrays."""
    i_pre = i_pre.astype(np.float64)
    f_pre = f_pre.astype(np.float64)
    vecI = np.log(1.0 / (1.0 + np.exp(-i_pre)) + EPS)
    vecF = np.log(1.0 / (1.0 + np.exp(-f_pre)) + EPS)
    Ic = vecI.reshape(B, NH, NCH, LC)
    Fc = vecF.reshape(B, NH, NCH, LC)
    vecB = np.cumsum(Fc, axis=-1)
    scaG = vecB[..., -1]
    vecA = scaG[..., None] - vecB + Ic

    ms = np.zeros((B, NH, NCH))
    dec = np.zeros((B, NH, NCH))
    m_new_arr = np.zeros((B, NH, NCH))
    m = np.zeros((B, NH))
    for c in range(NCH):
        amax = vecA[:, :, c, :].max(-1)
        m_new = np.maximum(scaG[:, :, c] + m, amax)
        ms[:, :, c] = m
        dec[:, :, c] = np.exp(scaG[:, :, c] + m - m_new)
        m_new_arr[:, :, c] = m_new
        m = m_new
    escale = np.exp(vecA - m_new_arr[..., None])

    mask = np.tril(np.ones((LC, LC), bool))
    logD = vecB[..., :, None] - vecB[..., None, :] + Ic[..., None, :]
    logD = np.where(mask, logD, -np.inf)
    m_intra = logD.max(-1)
    m_comb = np.maximum(vecB + ms[..., None], m_intra)
    Dp = np.where(mask, np.exp((Ic - vecB)[..., None, :] - ms[..., None, None]), 0.0)
    e2 = np.exp(-vecB - ms[..., None]) / QK_SCALE
    e3 = EPS * np.exp(m_comb - vecB - ms[..., None]) / QK_SCALE
    return dict(
        escale=escale.astype(np.float32), dec=dec.astype(np.float32),
        DpT=np.ascontiguousarray(Dp.transpose(0, 1, 2, 4, 3)).astype(np.float32),
        e2=e2.astype(np.float32), e3=e3.astype(np.float32))


def prep_weights(W_up, Wq, Wk, Wv, W_ig, W_fg, conv_w, conv_b, skip, W_down):
    """Host-side weight packing (same for all cores)."""
    FT = INNER // DH
    wupT = _bf(W_up.T)                                         # [512, 2048]
    wqkvT = _bf(np.concatenate([Wq.T, Wk.T, Wv.T], axis=1))    # [1024, 3072]
    wgT = _bf(np.concatenate([W_ig.T, W_fg.T], axis=1))        # [1024, 16]
    # diag[p, ft, t, col] = conv_w[ft*128+p, t] * (col == p)
    diag = np.zeros((DH, FT, KCONV, DH), np.float32)
    idx = np.arange(DH)
    cw = conv_w.reshape(FT, DH, KCONV)
    for ft in range(FT):
        for t in range(KCONV):
            diag[idx, ft, t, idx] = cw[ft, :, t]
    diag_i = _bf(diag.reshape(DH, FT * KCONV * DH))
    convb = np.ascontiguousarray(conv_b.reshape(FT, DH).T).astype(np.float32)
    skip_p = np.ascontiguousarray(skip.reshape(FT, DH).T).astype(np.float32)
    wdT = _bf(W_down.T)                                        # [1024, 512]
    return dict(wupT=wupT, wqkvT=wqkvT, wgT=wgT, diag_i=diag_i, convb=convb,
                skip_p=skip_p, wdT=wdT)


def build_a_inmaps(x, wp, b_ig, b_fg):
    """Per-core phase A input maps.  Core c = (b=c//4, quarter=c%4)."""
    in_maps = []
    for c in range(8):
        b, qt = c // 4, c % 4
        s0 = qt * TOK
        xs = x[b, :, :].T                                       # [512, S] view
        if s0 == 0:
            xt = np.concatenate([np.zeros((D, KCONV - 1), np.float32),
                                 xs[:, :TOK]], axis=1)
        else:
            xt = xs[:, s0 - (KCONV - 1): s0 + TOK]
        in_maps.append(dict(
            xt=_bf(xt), wupT=wp['wupT'], wqkvT=wp['wqkvT'], wgT=wp['wgT'],
            diag_i=wp['diag_i'], convb=wp['convb']))
    return in_maps


def assemble_a_outputs(a_results, b_ig, b_fg):
    """Concatenate per-core phase A outputs into full feature-major tensors.

    Returns q_t,k_t,v_t,xm_t,xog_t as [B, INNER, S] (bf16-valued f32 arrays
    stay in ml_dtypes.bfloat16) and i_pre,f_pre [B, NH, S] f32 (bias added)."""
    def cat(name):
        return np.stack([
            np.concatenate([a_results[b * 4 + qt][name] for qt in range(4)], axis=1)
            for b in range(B)])
    q_t, k_t, v_t = cat('q_o'), cat('k_o'), cat('v_o')          # [B, INNER, S] bf16
    xm_t, xog_t = cat('xm_o'), cat('xog_o')
    gates = cat('gates_o').astype(np.float32)                   # [B, 16, S]
    i_pre = gates[:, :NH, :] + np.asarray(b_ig, np.float32)[None, :, None]
    f_pre = gates[:, NH:, :] + np.asarray(b_fg, np.float32)[None, :, None]
    return q_t, k_t, v_t, xm_t, xog_t, i_pre, f_pre


def build_b_inmaps(q_t, k_t, v_t, g):
    """Per-core phase B inputs.  Core c handles units (b, 2h) where
    b = c // 4, heads (2*(c%4), 2*(c%4)+1)."""
    NW = 132
    in_maps = []
    for c in range(8):
        b, hp = c // 4, c % 4
        m = {}
        for u in range(NUNIT):
            h = 2 * hp + u
            rs = slice(h * DH, (h + 1) * DH)
            qT = np.ascontiguousarray(q_t[b, rs, :])            # [128, S] bf16
            kT = np.ascontiguousarray(k_t[b, rs, :])
            k_tok = k_t[b, rs, :].T.astype(np.float32)          # [S, 128]
            esc = g['escale'][b, h].reshape(S)                  # [S]
            kesc = _bf(k_tok * esc[:, None])
            vone = np.zeros((S, NW), np.float32)
            vone[:, :DH] = v_t[b, rs, :].T.astype(np.float32)
            vone[:, DH] = 1.0
            DpT = _bf(g['DpT'][b, h].reshape(S, DH))            # [NCH*128(j), 128(l)]
            m[f"qT{u}"] = qT
            m[f"kT{u}"] = kT
            m[f"kesc{u}"] = kesc
            m[f"vone{u}"] = _bf(vone)
            m[f"DpT{u}"] = DpT
            m[f"dec{u}"] = np.ascontiguousarray(
                np.broadcast_to(g['dec'][b, h][None, :], (DH, NCH)).astype(np.float32))
            m[f"e2{u}"] = np.ascontiguousarray(g['e2'][b, h].T.astype(np.float32))
            m[f"e3{u}"] = np.ascontiguousarray(g['e3'][b, h].T.astype(np.float32))
        in_maps.append(m)
    return in_maps


def build_c_inmaps(b_results, xm_t, xog_t, wp):
    """Assemble h from phase B and build per-core phase C inputs."""
    # h per (b, h): [S, 128] f32 -> full feature-major h_t [B, INNER, S] bf16
    h_t = np.empty((B, INNER, S), np.float32)
    for c in range(8):
        b, hp = c // 4, c % 4
        for u in range(NUNIT):
            h = 2 * hp + u
            h_t[b, h * DH:(h + 1) * DH, :] = b_results[c][f"h{u}"].T
    in_maps = []
    for c in range(8):
        b, qt = c // 4, c % 4
        ts = slice(qt * TOK, (qt + 1) * TOK)
        in_maps.append(dict(
            h_i=_bf(h_t[b, :, ts]),
            xm_i=np.ascontiguousarray(xm_t[b, :, ts]),
            xog_i=np.ascontiguousarray(xog_t[b, :, ts]),
            skip_i=wp['skip_p'], wdT=wp['wdT']))
    return in_maps


def assemble_output(c_results):
    out = np.empty((B, S, D), np.float32)
    for c in range(8):
        b, qt = c // 4, c % 4
        out[b, qt * TOK:(qt + 1) * TOK, :] = c_results[c]['out_o'].T
    return out


# ------------------------------------------------------------------ entry ----
from concourse.bass_utils import run_bass_kernel_spmd as _run_spmd

_CACHE = {}


def _programs():
    if 'a' not in _CACHE:
        _CACHE['a'] = build_phase_a()
        _CACHE['b'] = build_phase_b()
        _CACHE['c'] = build_phase_c()
    return _CACHE['a'], _CACHE['b'], _CACHE['c']


def kernel(x, W_up, Wq, Wk, Wv, W_ig, b_ig, W_fg, b_fg, conv_w, conv_b, skip,
           W_down):
    x = np.asarray(x, np.float32)
    nc_a, nc_b, nc_c = _programs()
    cores = list(range(8))
    wp = prep_weights(W_up, Wq, Wk, Wv, W_ig, W_fg, conv_w, conv_b, skip, W_down)
    a_maps = build_a_inmaps(x, wp, b_ig, b_fg)
    ra = _run_spmd(nc_a, a_maps, core_ids=cores).results
    q_t, k_t, v_t, xm_t, xog_t, i_pre, f_pre = assemble_a_outputs(ra, b_ig, b_fg)
    g = host_gate_math(i_pre, f_pre)
    b_maps = build_b_inmaps(q_t, k_t, v_t, g)
    rb = _run_spmd(nc_b, b_maps, core_ids=cores).results
    c_maps = build_c_inmaps(rb, xm_t, xog_t, wp)
    rc = _run_spmd(nc_c, c_maps, core_ids=cores).results
    return assemble_output(rc)



# revision 9
# speedup vs baseline: 1.3609x; 1.3609x over previous
"""Trainium2 Bass kernel for nn_ChunkwiseMLSTM (B=2, S=8192, D=512, INNER=1024, NH=8).

kernel(**inputs) -> np.ndarray [2, 8192, 512] f32.

Three SPMD launches on 8 NeuronCores:
  A: token-sharded projections (up-proj via fp8 hi/lo DoubleRow, causal conv,
     silu, q/k/v via fp8 hi/lo DoubleRow, gate pre-acts, silu(x_og))
  B: head-sharded chunkwise mLSTM (LC=128 chunks; f32 state, bf16 matmuls,
     DpT folded into k on host, engine-balanced elementwise work)
  C: token-sharded output gating + down-projection
Host between launches: gate-derived scan scalars (O(B*NH*S) work), weight
packing, and resharding.

fp8 hi/lo trick: y*64 = sum_k Wh@xh + Wh@xl + Wl@xh with Wh=fp8(64W),
Wl=fp8(64W-Wh), xh=fp8(x), xl=fp8(x-xh).  All three terms share the scale so
they accumulate in one PSUM group; DoubleRow packs 2 pairs per instruction
(0.5 cyc/row vs bf16's 1.0).  Residual error ~0.2% (better than bf16 matmul
inputs); the dropped Wl@xl term is ~0.1%.
"""
import os
os.environ.setdefault("JAX_COMPILATION_CACHE_DIR",
                      os.path.expanduser("~/.cache/jax_bass_cache"))
os.environ.setdefault("JAX_PERSISTENT_CACHE_MIN_ENTRY_SIZE_BYTES", "0")
os.environ.setdefault("JAX_PERSISTENT_CACHE_MIN_COMPILE_TIME_SECS", "0")

import sys
if '/opt/trn_rl_repo' not in sys.path:
    sys.path.insert(0, '/opt/trn_rl_repo')

import numpy as np
import ml_dtypes

import concourse.bass as bass
import concourse.tile as tile
from concourse import mybir, bacc

F32 = mybir.dt.float32
BF16 = mybir.dt.bfloat16
FP8 = mybir.dt.float8e4
AF = mybir.ActivationFunctionType
OP = mybir.AluOpType
DR = mybir.MatmulPerfMode.DoubleRow

B, S, D = 2, 8192, 512
INNER, NH, KCONV = 1024, 8, 4
DH = 128
EPS = 1e-6
LC = 128           # chunk length used on device (math is chunk-size invariant)
NCH = S // LC      # 64
QK_SCALE = DH ** -0.5
TOK = S // 4       # tokens per core in phases A/C = 2048
TH = TOK + (KCONV - 1)   # 2051 with conv halo
NUNIT = 2          # (b,h) units per core in phase B
NW = 132           # padded width for [C|n] and [v|1]
SW = 64.0          # weight pre-scale for fp8 hi/lo

A_OUT_NAMES = ['q_o', 'k_o', 'v_o', 'xm_o', 'xg_o', 'gates_o']


def _bf(x):
    return np.ascontiguousarray(np.asarray(x, np.float32).astype(ml_dtypes.bfloat16))


def _f8(x):
    return np.ascontiguousarray(
        np.asarray(x, np.float32).astype(ml_dtypes.float8_e4m3))


def _hilo(x):
    """Return (hi, lo) fp8e4m3 split of float array x (hi+lo ~= x)."""
    x = np.asarray(x, np.float32)
    hi = x.astype(ml_dtypes.float8_e4m3)
    lo = (x - hi.astype(np.float32)).astype(ml_dtypes.float8_e4m3)
    return hi, lo


def new_nc():
    return bacc.Bacc(None, target_bir_lowering=False, debug=False)


# ---------------------------------------------------------------- phase A ----
def build_phase_a():
    nc = new_nc()
    KT_UP = D // DH          # 4 k-tiles for up-proj
    MT_UP = 2 * INNER // DH  # 16 m-tiles
    FT = INNER // DH         # 8 feature tiles of the mlstm half
    KT_IN = INNER // DH      # 8 k-tiles over INNER
    MT_QKV = 3 * INNER // DH # 24

    # fp8 hi/lo packed inputs. xhl: [p, kt, {hi,lo}, t]; w: [p, kt, {lo,hi}, m]
    xhl_i = nc.dram_tensor("xhl", [DH, KT_UP * 2 * TH], FP8, kind="ExternalInput")
    wup_i = nc.dram_tensor("wup_hl", [DH, KT_UP * 2 * (2 * INNER)], FP8, kind="ExternalInput")
    wqkv_i = nc.dram_tensor("wqkv_hl", [DH, MT_QKV * KT_IN * 2 * DH], FP8, kind="ExternalInput")
    wgT = nc.dram_tensor("wgT", [INNER, 2 * NH], BF16, kind="ExternalInput")   # [Wig.T | Wfg.T]
    # host-built diagonal conv weight tiles: diag[p, ft, t, col] = conv_w[ft*128+p, t] * (col==p)
    diag_i = nc.dram_tensor("diag_i", [DH, FT * KCONV * DH], BF16, kind="ExternalInput")
    convb = nc.dram_tensor("convb", [DH, FT], F32, kind="ExternalInput")

    q_o = nc.dram_tensor("q_o", [INNER, TOK], BF16, kind="ExternalOutput")
    k_o = nc.dram_tensor("k_o", [INNER, TOK], BF16, kind="ExternalOutput")
    v_o = nc.dram_tensor("v_o", [INNER, TOK], BF16, kind="ExternalOutput")
    xm_o = nc.dram_tensor("xm_o", [INNER, TOK], BF16, kind="ExternalOutput")
    xg_o = nc.dram_tensor("xg_o", [INNER, TOK], BF16, kind="ExternalOutput")
    gates_o = nc.dram_tensor("gates_o", [2 * NH, TOK], F32, kind="ExternalOutput")

    RSW = 1.0 / SW
    # token n-tiles over TH (matmul N <= 512)
    N_SL = [(i * 512, min(512, TH - i * 512)) for i in range((TH + 511) // 512)]
    N_SL_OUT = [(i * 512, 512) for i in range(TOK // 512)]

    def dr_group(pt, w_sb, x_sb, kt_n, m_sl, n_sl):
        """Emit the fp8 hi/lo DoubleRow accumulation group into psum pt.

        w_sb: [p, kt, 2(lo,hi), M]; x_sb: [p, kt, 2(hi,lo), N]."""
        n0, nn = n_sl
        first = True
        # hi x hi: k-tile pairs, picking the hi slot of each
        for k in range(0, kt_n, 2):
            nc.tensor.matmul(
                pt[:, :nn],
                w_sb[:, k:k + 2, 1, m_sl],
                x_sb[:, k:k + 2, 0, n0:n0 + nn],
                start=first, stop=False, perf_mode=DR)
            first = False
        # cross terms: (Wl,xh) + (Wh,xl) per k-tile
        for k in range(kt_n):
            nc.tensor.matmul(
                pt[:, :nn],
                w_sb[:, k, :, m_sl],
                x_sb[:, k, :, n0:n0 + nn],
                start=False, stop=(k == kt_n - 1), perf_mode=DR)

    with tile.TileContext(nc) as tc, \
         tc.tile_pool(name="const", bufs=1) as const, \
         tc.tile_pool(name="big", bufs=1) as big, \
         tc.tile_pool(name="wpool", bufs=3) as wpool, \
         tc.tile_pool(name="ev", bufs=4) as ev, \
         tc.tile_pool(name="ps", bufs=4, space="PSUM") as ps:
        if True:
            # --- load x (fp8 hi/lo) and W_up (fp8 hi/lo)
            xhl_sb = big.tile([DH, KT_UP, 2, TH], FP8)
            nc.sync.dma_start(
                out=xhl_sb,
                in_=xhl_i[:].rearrange("p (kt s t) -> p kt s t", kt=KT_UP, s=2))
            wup_sb = big.tile([DH, KT_UP, 2, 2 * INNER], FP8)
            nc.sync.dma_start(
                out=wup_sb,
                in_=wup_i[:].rearrange("p (kt s m) -> p kt s m", kt=KT_UP, s=2))

            # --- up-projection: x_pre (mlstm half, bf16) and xg = silu(x_og)
            xpre_sb = big.tile([DH, FT, TH], BF16)
            for m in range(MT_UP):
                mlstm_half = m < FT
                for (n0, nn) in (N_SL if mlstm_half else N_SL_OUT):
                    pt = ps.tile([DH, 512], F32)
                    off = 0 if mlstm_half else KCONV - 1
                    dr_group(pt, wup_sb, xhl_sb, KT_UP,
                             slice(m * DH, (m + 1) * DH), (off + n0, nn))
                    if mlstm_half:
                        nc.scalar.activation(
                            xpre_sb[:, m, n0:n0 + nn], pt[:, :nn], AF.Copy,
                            scale=RSW)
                    else:
                        # xg = silu(x_og) = (pt/SW) * sigmoid(pt/SW)
                        sg_t = ev.tile([DH, 512], BF16, tag="sg")
                        nc.scalar.activation(
                            sg_t[:, :nn], pt[:, :nn], AF.Sigmoid, scale=RSW)
                        xg_t = ev.tile([DH, 512], BF16, tag="xg")
                        nc.vector.scalar_tensor_tensor(
                            xg_t[:, :nn], pt[:, :nn], RSW, sg_t[:, :nn],
                            OP.mult, OP.mult)
                        nc.sync.dma_start(
                            out=xg_o[(m - FT) * DH:(m - FT + 1) * DH, n0:n0 + nn],
                            in_=xg_t[:, :nn])

            # --- causal depthwise conv as 4 diag matmuls + SiLU (bf16)
            convb_sb = const.tile([DH, FT], F32)
            nc.sync.dma_start(out=convb_sb, in_=convb[:])
            diag = const.tile([DH, FT, KCONV, DH], BF16)
            nc.sync.dma_start(
                out=diag, in_=diag_i[:].rearrange("p (ft t c) -> p ft t c", ft=FT, t=KCONV))

            xm_sb = big.tile([DH, FT, TOK], BF16)
            xmhl_sb = big.tile([DH, KT_IN, 2, TOK], FP8)
            for ft in range(FT):
                for (n0, nn) in N_SL_OUT:
                    pt = ps.tile([DH, 512], F32)
                    for t in range(KCONV):
                        nc.tensor.matmul(
                            pt[:, :nn],
                            diag[:, ft, t, :],
                            xpre_sb[:, ft, n0 + t: n0 + t + nn],
                            start=(t == 0), stop=(t == KCONV - 1))
                    # silu(y+b) = (y+b) * sigmoid(y+b)
                    sg_t = ev.tile([DH, 512], BF16, tag="sgc")
                    nc.scalar.activation(
                        sg_t[:, :nn], pt[:, :nn], AF.Sigmoid,
                        bias=convb_sb[:, ft:ft + 1], scale=1.0)
                    nc.vector.scalar_tensor_tensor(
                        xm_sb[:, ft, n0:n0 + nn], pt[:, :nn], convb_sb[:, ft:ft + 1],
                        sg_t[:, :nn], OP.add, OP.mult)
                    # fp8 hi/lo split of xm for the qkv matmuls
                    nc.scalar.activation(
                        xmhl_sb[:, ft, 0, n0:n0 + nn], xm_sb[:, ft, n0:n0 + nn],
                        AF.Copy)
                    nc.vector.tensor_tensor(
                        xmhl_sb[:, ft, 1, n0:n0 + nn], xm_sb[:, ft, n0:n0 + nn],
                        xmhl_sb[:, ft, 0, n0:n0 + nn], OP.subtract)
            nc.sync.dma_start(out=xm_o[:].rearrange("(ft p) t -> p ft t", p=DH), in_=xm_sb)

            # --- q/k/v projections (streamed fp8 hi/lo weights)
            qkv_outs = [q_o, k_o, v_o]
            WBLK = KT_IN * 2 * DH
            for m in range(MT_QKV):
                w_sb = wpool.tile([DH, KT_IN, 2, DH], FP8)
                nc.sync.dma_start(
                    out=w_sb,
                    in_=wqkv_i[:, m * WBLK:(m + 1) * WBLK].rearrange(
                        "p (kt s m) -> p kt s m", kt=KT_IN, s=2))
                out_t = qkv_outs[m // FT]
                mf = m % FT
                for ni, (n0, nn) in enumerate(N_SL_OUT):
                    pt = ps.tile([DH, 512], F32)
                    dr_group(pt, w_sb, xmhl_sb, KT_IN, slice(None), (n0, nn))
                    ev_t = ev.tile([DH, 512], BF16)
                    if (m + ni) % 2 == 0:
                        nc.scalar.activation(ev_t[:, :nn], pt[:, :nn], AF.Copy,
                                             scale=RSW)
                    else:
                        nc.vector.tensor_scalar_mul(ev_t[:, :nn], pt[:, :nn], RSW)
                    nc.sync.dma_start(
                        out=out_t[mf * DH:(mf + 1) * DH, n0:n0 + nn], in_=ev_t[:, :nn])

            # gates: [16, TOK] bf16 matmul (tiny, precision-sensitive)
            wg_sb = const.tile([DH, KT_IN, 2 * NH], BF16)
            nc.sync.dma_start(out=wg_sb, in_=wgT[:].rearrange("(kt p) m -> p kt m", p=DH))
            for (n0, nn) in N_SL_OUT:
                pt = ps.tile([2 * NH, 512], F32)
                for kt in range(KT_IN):
                    nc.tensor.matmul(
                        pt[:, :nn], wg_sb[:, kt, :], xm_sb[:, kt, n0:n0 + nn],
                        start=(kt == 0), stop=(kt == KT_IN - 1))
                gv = ev.tile([2 * NH, 512], F32, tag="gv")
                nc.vector.tensor_copy(gv[:, :nn], pt[:, :nn])
                nc.sync.dma_start(out=gates_o[:, n0:n0 + nn], in_=gv[:, :nn])
    nc.compile()
    return nc


# ---------------------------------------------------------------- phase B ----
def build_phase_b(use_f32r=False):
    """use_f32r: feed the f32 state directly to the num matmul as float32r —
    REJECTED by walrus (no 32/16-bit matmul input mixing); keep False.
    False: cast state to bf16 on Pool (SBUF->SBUF, legal for GPSIMD)."""
    nc = new_nc()
    F32R = mybir.dt.float32r
    SB = 4   # mm1/mask batch (chunks)
    HB = 8   # chunks per h DMA batch
    ins = {}
    outs = {}
    mask_i = nc.dram_tensor("mask", [DH, SB * DH], BF16, kind="ExternalInput")
    for u in range(NUNIT):
        ins[f"qT{u}"] = nc.dram_tensor(f"qT{u}", [DH, S], BF16, kind="ExternalInput")
        ins[f"kT{u}"] = nc.dram_tensor(f"kT{u}", [DH, S], BF16, kind="ExternalInput")
        # p-major packed: [p, c, e] with token = c*128+p
        ins[f"kesc{u}"] = nc.dram_tensor(f"kesc{u}", [DH, NCH * DH], BF16, kind="ExternalInput")
        ins[f"vone{u}"] = nc.dram_tensor(f"vone{u}", [DH, NCH * NW], BF16, kind="ExternalInput")
        ins[f"dec{u}"] = nc.dram_tensor(f"dec{u}", [DH, NCH], F32, kind="ExternalInput")
        ins[f"e2{u}"] = nc.dram_tensor(f"e2{u}", [DH, NCH], F32, kind="ExternalInput")
        # p-major packed h out: [p, c, e] bf16
        outs[f"h{u}"] = nc.dram_tensor(f"h{u}", [DH, NCH * DH], BF16, kind="ExternalOutput")

    with tile.TileContext(nc) as tc, \
         tc.tile_pool(name="big", bufs=1) as big, \
         tc.tile_pool(name="work", bufs=2) as work, \
         tc.tile_pool(name="dwork", bufs=3) as dwork, \
         tc.tile_pool(name="state", bufs=2) as state, \
         tc.tile_pool(name="cbp", bufs=3) as cbp, \
         tc.tile_pool(name="ps_s", bufs=1, space="PSUM") as ps_s, \
         tc.tile_pool(name="ps_num", bufs=2, space="PSUM") as ps_num, \
         tc.tile_pool(name="ps_u", bufs=1, space="PSUM") as ps_u:
        if True:
            mask = big.tile([DH, SB * DH], BF16, name="mask")
            nc.sync.dma_start(out=mask, in_=mask_i[:])
            T = {}
            for u in range(NUNIT):
                T[u] = dict(
                    qT=big.tile([DH, S], BF16, name=f"qT{u}"),
                    kT=big.tile([DH, S], BF16, name=f"kT{u}"),
                    kesc=big.tile([DH, NCH, DH], BF16, name=f"kesc{u}"),
                    vone=big.tile([DH, NCH, NW], BF16, name=f"vone{u}"),
                    dec=big.tile([DH, NCH], F32, name=f"dec{u}"),
                    e2=big.tile([DH, NCH], F32, name=f"e2{u}"),
                    hbuf=big.tile([DH, NCH, DH], BF16, name=f"hbuf{u}"),
                )
                t = T[u]
                nc.sync.dma_start(out=t['qT'], in_=ins[f"qT{u}"][:])
                nc.sync.dma_start(out=t['kT'], in_=ins[f"kT{u}"][:])
                nc.sync.dma_start(
                    out=t['kesc'],
                    in_=ins[f"kesc{u}"][:].rearrange("p (c e) -> p c e", c=NCH))
                nc.sync.dma_start(
                    out=t['vone'],
                    in_=ins[f"vone{u}"][:].rearrange("p (c e) -> p c e", c=NCH))
                nc.sync.dma_start(out=t['dec'], in_=ins[f"dec{u}"][:])
                nc.sync.dma_start(out=t['e2'], in_=ins[f"e2{u}"][:])
                cf = state.tile([DH, NW], F32, tag=f"Cf{u}")
                nc.vector.memset(cf[:], 0.0)
                t['cf'] = cf
                if not use_f32r:
                    cb = cbp.tile([DH, NW], BF16, tag=f"Cb{u}")
                    nc.gpsimd.memset(cb[:], 0.0)
                    t['cb'] = cb

            def chunk_body(u, c):
                t = T[u]
                csl = slice(c * LC, (c + 1) * LC)
                cq = c % SB
                if cq == 0:
                    # mm1 batch: S_T[j,l] for chunks c..c+SB-1 (state-independent)
                    s4 = ps_s.tile([DH, SB * DH], F32, tag=f"s4_{u}")
                    for cc in range(SB):
                        ccsl = slice((c + cc) * LC, (c + cc + 1) * LC)
                        nc.tensor.matmul(
                            s4[:, cc * DH:(cc + 1) * DH],
                            t['kT'][:, ccsl], t['qT'][:, ccsl],
                            start=True, stop=True)
                    # sp = S_T * mask (upper-tri in [j,l]) -> bf16   [DVE]
                    sp4 = work.tile([DH, SB * DH], BF16, tag=f"sp4_{u}")
                    nc.vector.tensor_tensor(sp4[:], s4[:], mask[:], OP.mult)
                    t['sp4'] = sp4
                # num = q_c^T-mm @ [C|n]  +  Sp^T @ [v|1]   (double-wide tile)
                if c % 2 == 0:
                    t['num2'] = ps_num.tile([DH, 2 * NW], F32, tag=f"num2_{u}",
                                            name=f"num2_{u}_{c}")
                num2 = t['num2']
                half = (c % 2) * NW
                cterm = (t['cf'][:].bitcast(F32R) if use_f32r else t['cb'][:])
                nc.tensor.matmul(num2[:, half:half + NW], t['qT'][:, csl],
                                 cterm, start=True, stop=False)
                nc.tensor.matmul(num2[:, half:half + NW],
                                 t['sp4'][:, cq * DH:(cq + 1) * DH],
                                 t['vone'][:, c, :], start=False, stop=True)
                # mm2: U = kesc_c^T @ [v|1]
                u_ps = ps_u.tile([DH, NW], F32, tag=f"u_{u}")
                nc.tensor.matmul(u_ps[:], t['kesc'][:, c, :], t['vone'][:, c, :],
                                 start=True, stop=True)
                # C_new = C*dec + U  (ping-pong)  [DVE]
                cf_new = state.tile([DH, NW], F32, tag=f"Cf{u}")
                nc.vector.scalar_tensor_tensor(
                    cf_new[:], t['cf'][:], t['dec'][:, c:c + 1], u_ps[:],
                    OP.mult, OP.add)
                t['cf'] = cf_new
                if not use_f32r:
                    cb_new = cbp.tile([DH, NW], BF16, tag=f"Cb{u}")
                    nc.gpsimd.tensor_copy(cb_new[:], cf_new[:])
                    t['cb'] = cb_new
                if c % 2 == 1:
                    # den pair: |num[:,128]| max e2 -> reciprocal   (2 chunks)
                    dcols = num2[:, DH:DH + NW + 1:NW]          # [128, 2]
                    absd = dwork.tile([DH, 2], F32, tag=f"absd{u}")
                    nc.scalar.activation(absd[:], dcols, AF.Abs)
                    den = dwork.tile([DH, 2], F32, tag=f"den{u}")
                    nc.vector.tensor_tensor(den[:], absd[:],
                                            t['e2'][:, c - 1:c + 1], OP.max)
                    rden = dwork.tile([DH, 2], F32, tag=f"rden{u}")
                    nc.vector.reciprocal(rden[:], den[:])
                    # h = num[:, :128] * rden   [ACT copy w/ per-partition scale]
                    nc.scalar.activation(t['hbuf'][:, c - 1, :], num2[:, :DH],
                                         AF.Copy, bias=0.0, scale=rden[:, 0:1])
                    nc.scalar.activation(t['hbuf'][:, c, :], num2[:, NW:NW + DH],
                                         AF.Copy, bias=0.0, scale=rden[:, 1:2])
                if c % HB == HB - 1:
                    c0 = c - (HB - 1)
                    nc.sync.dma_start(
                        out=outs[f"h{u}"][:, c0 * DH:(c + 1) * DH],
                        in_=t['hbuf'][:, c0:c + 1, :])

            for c in range(NCH):
                for u in range(NUNIT):
                    chunk_body(u, c)
    nc.compile()
    return nc


# ---------------------------------------------------------------- phase C ----
def build_phase_c():
    nc = new_nc()
    h_i = nc.dram_tensor("h_i", [INNER, TOK], BF16, kind="ExternalInput")
    xm_i = nc.dram_tensor("xm_i", [INNER, TOK], BF16, kind="ExternalInput")
    xg_i = nc.dram_tensor("xg_i", [INNER, TOK], BF16, kind="ExternalInput")
    skip_i = nc.dram_tensor("skip_i", [DH, INNER // DH], F32, kind="ExternalInput")
    wdT = nc.dram_tensor("wdT", [INNER, D], BF16, kind="ExternalInput")
    out_o = nc.dram_tensor("out_o", [D, TOK], F32, kind="ExternalOutput")

    FT = INNER // DH   # 8
    MT = D // DH       # 4
    N_SL = [(i * 512, 512) for i in range(TOK // 512)]
    with tile.TileContext(nc) as tc, \
         tc.tile_pool(name="big", bufs=1) as big, \
         tc.tile_pool(name="ev", bufs=4) as ev, \
         tc.tile_pool(name="outp", bufs=2) as outp, \
         tc.tile_pool(name="ps", bufs=4, space="PSUM") as ps:
        if True:
            skip_sb = big.tile([DH, FT], F32)
            nc.sync.dma_start(out=skip_sb, in_=skip_i[:])
            wd_sb = big.tile([DH, FT, D], BF16)
            nc.sync.dma_start(out=wd_sb, in_=wdT[:].rearrange("(ft p) m -> p ft m", p=DH))

            h_sb = big.tile([DH, FT, TOK], BF16)
            xm_sb = big.tile([DH, FT, TOK], BF16)
            xg_sb = big.tile([DH, FT, TOK], BF16)
            hg_sb = big.tile([DH, FT, TOK], BF16)
            # per-ft loads so gating pipelines behind DMA
            for ft in range(FT):
                fsl = slice(ft * DH, (ft + 1) * DH)
                nc.sync.dma_start(
                    out=h_sb[:, ft, :], in_=h_i[fsl, :])
                nc.sync.dma_start(
                    out=xm_sb[:, ft, :], in_=xm_i[fsl, :])
                nc.sync.dma_start(
                    out=xg_sb[:, ft, :], in_=xg_i[fsl, :])
                hs = ev.tile([DH, TOK], BF16, tag="hs")
                nc.vector.scalar_tensor_tensor(
                    hs[:], xm_sb[:, ft, :], skip_sb[:, ft:ft + 1], h_sb[:, ft, :],
                    OP.mult, OP.add)
                nc.vector.tensor_tensor(hg_sb[:, ft, :], hs[:], xg_sb[:, ft, :],
                                        OP.mult)

            for m in range(MT):
                ot = outp.tile([DH, TOK], F32, tag="ot")
                for (n0, nn) in N_SL:
                    pt = ps.tile([DH, 512], F32)
                    for kt in range(FT):
                        nc.tensor.matmul(
                            pt[:, :nn], wd_sb[:, kt, m * DH:(m + 1) * DH],
                            hg_sb[:, kt, n0:n0 + nn],
                            start=(kt == 0), stop=(kt == FT - 1))
                    nc.scalar.copy(ot[:, n0:n0 + nn], pt[:, :nn])
                nc.sync.dma_start(out=out_o[m * DH:(m + 1) * DH, :], in_=ot[:])
    nc.compile()
    return nc


# ------------------------------------------------------------- host glue ----
def host_gate_math(i_pre, f_pre):
    """i_pre, f_pre: [B, NH, S] f32.  Returns dict of f32 arrays."""
    i_pre = i_pre.astype(np.float64)
    f_pre = f_pre.astype(np.float64)
    vecI = np.log(1.0 / (1.0 + np.exp(-i_pre)) + EPS)
    vecF = np.log(1.0 / (1.0 + np.exp(-f_pre)) + EPS)
    Ic = vecI.reshape(B, NH, NCH, LC)
    Fc = vecF.reshape(B, NH, NCH, LC)
    vecB = np.cumsum(Fc, axis=-1)
    scaG = vecB[..., -1]
    vecA = scaG[..., None] - vecB + Ic

    ms = np.zeros((B, NH, NCH))
    dec = np.zeros((B, NH, NCH))
    m_new_arr = np.zeros((B, NH, NCH))
    m = np.zeros((B, NH))
    for c in range(NCH):
        amax = vecA[:, :, c, :].max(-1)
        m_new = np.maximum(scaG[:, :, c] + m, amax)
        ms[:, :, c] = m
        dec[:, :, c] = np.exp(scaG[:, :, c] + m - m_new)
        m_new_arr[:, :, c] = m_new
        m = m_new
    escale = np.exp(vecA - m_new_arr[..., None])

    # cb: the (token-j) column factor of the decay matrix, folded into k.
    cb = np.exp(Ic - vecB - ms[..., None])            # [B, NH, NCH, LC]
    e2 = np.exp(-vecB - ms[..., None]) / QK_SCALE     # [B, NH, NCH, LC]
    return dict(
        escale=escale.astype(np.float32), dec=dec.astype(np.float32),
        cb=cb.astype(np.float32), e2=e2.astype(np.float32))


def prep_weights(W_up, Wq, Wk, Wv, W_ig, W_fg, conv_w, conv_b, skip, W_down):
    """Host-side weight packing (same for all cores)."""
    FT = INNER // DH
    KT_UP = D // DH
    KT_IN = INNER // DH
    MT_QKV = 3 * INNER // DH

    # up-proj fp8 hi/lo: [p, kt, {lo,hi}, m] flattened
    wupT = np.asarray(W_up, np.float32).T * SW           # [512, 2048]
    hi, lo = _hilo(wupT)
    wup_hl = np.stack([lo.reshape(KT_UP, DH, 2 * INNER),
                       hi.reshape(KT_UP, DH, 2 * INNER)], axis=2)  # [kt, p, 2, m]
    wup_hl = np.ascontiguousarray(wup_hl.transpose(1, 0, 2, 3)).reshape(DH, -1)

    # qkv fp8 hi/lo: per m-tile blocks [p, kt, {lo,hi}, 128]
    wqkvT = np.concatenate(
        [np.asarray(W, np.float32).T for W in (Wq, Wk, Wv)], axis=1) * SW  # [1024, 3072]
    hi, lo = _hilo(wqkvT)
    # [kt, p, m] -> [m-tile, kt, p, 2, 128] -> [p, m-tile, kt, 2, 128]
    hi = hi.reshape(KT_IN, DH, MT_QKV, DH)
    lo = lo.reshape(KT_IN, DH, MT_QKV, DH)
    wqkv_hl = np.stack([lo, hi], axis=3)                  # [kt, p, mt, 2, 128]
    wqkv_hl = np.ascontiguousarray(
        wqkv_hl.transpose(1, 2, 0, 3, 4)).reshape(DH, -1)  # [p, mt*kt*2*128]

    wgT = _bf(np.concatenate([np.asarray(W_ig, np.float32).T,
                              np.asarray(W_fg, np.float32).T], axis=1))  # [1024, 16]
    # diag[p, ft, t, col] = conv_w[ft*128+p, t] * (col == p)
    diag = np.zeros((DH, FT, KCONV, DH), np.float32)
    idx = np.arange(DH)
    cw = np.asarray(conv_w, np.float32).reshape(FT, DH, KCONV)
    for ft in range(FT):
        for t in range(KCONV):
            diag[idx, ft, t, idx] = cw[ft, :, t]
    diag_i = _bf(diag.reshape(DH, FT * KCONV * DH))
    convb = np.ascontiguousarray(
        np.asarray(conv_b, np.float32).reshape(FT, DH).T)
    skip_p = np.ascontiguousarray(
        np.asarray(skip, np.float32).reshape(FT, DH).T)
    wdT = _bf(np.asarray(W_down, np.float32).T)            # [1024, 512]
    return dict(wup_hl=wup_hl, wqkv_hl=wqkv_hl, wgT=wgT, diag_i=diag_i,
                convb=convb, skip_p=skip_p, wdT=wdT)


def build_a_inmaps(x, wp, b_ig, b_fg):
    """Per-core phase A input maps.  Core c = (b=c//4, quarter=c%4)."""
    KT_UP = D // DH
    in_maps = []
    for c in range(8):
        b, qt = c // 4, c % 4
        s0 = qt * TOK
        xs = np.asarray(x[b], np.float32).T                 # [512, S]
        if s0 == 0:
            xt = np.concatenate([np.zeros((D, KCONV - 1), np.float32),
                                 xs[:, :TOK]], axis=1)
        else:
            xt = xs[:, s0 - (KCONV - 1): s0 + TOK]
        hi, lo = _hilo(xt)
        xhl = np.stack([hi.reshape(KT_UP, DH, TH),
                        lo.reshape(KT_UP, DH, TH)], axis=2)  # [kt, p, 2, t]
        xhl = np.ascontiguousarray(xhl.transpose(1, 0, 2, 3)).reshape(DH, -1)
        in_maps.append(dict(
            xhl=xhl, wup_hl=wp['wup_hl'], wqkv_hl=wp['wqkv_hl'], wgT=wp['wgT'],
            diag_i=wp['diag_i'], convb=wp['convb']))
    return in_maps


def assemble_a_outputs(a_results, b_ig, b_fg):
    """Concatenate per-core phase A outputs into full feature-major tensors."""
    def cat(name):
        return np.stack([
            np.concatenate([a_results[b * 4 + qt][name] for qt in range(4)], axis=1)
            for b in range(B)])
    q_t, k_t, v_t = cat('q_o'), cat('k_o'), cat('v_o')          # [B, INNER, S] bf16
    xm_t, xg_t = cat('xm_o'), cat('xg_o')
    gates = cat('gates_o').astype(np.float32)                   # [B, 16, S]
    i_pre = gates[:, :NH, :] + np.asarray(b_ig, np.float32)[None, :, None]
    f_pre = gates[:, NH:, :] + np.asarray(b_fg, np.float32)[None, :, None]
    return q_t, k_t, v_t, xm_t, xg_t, i_pre, f_pre


def _pmajor(x_tok, width):
    """[S, width] -> p-major packed [128, NCH*width]."""
    return np.ascontiguousarray(
        x_tok.reshape(NCH, DH, width).transpose(1, 0, 2)).reshape(DH, -1)


def build_b_inmaps(q_t, k_t, v_t, g):
    """Per-core phase B inputs.  Core c handles units (b, 2h) where
    b = c // 4, heads (2*(c%4), 2*(c%4)+1)."""
    mask = _bf(np.tile(np.triu(np.ones((DH, DH), np.float32)), (1, 4)))  # keep l >= j
    in_maps = []
    for c in range(8):
        b, hp = c // 4, c % 4
        m = {'mask': mask}
        for u in range(NUNIT):
            h = 2 * hp + u
            rs = slice(h * DH, (h + 1) * DH)
            k_f = k_t[b, rs, :].astype(np.float32)              # [128, S]
            qT = np.ascontiguousarray(q_t[b, rs, :])
            kT = _bf(k_f * g['cb'][b, h].reshape(S)[None, :])   # fold DpT col factor
            k_tok = k_f.T                                       # [S, 128]
            esc = g['escale'][b, h].reshape(S)
            kesc = _bf(k_tok * esc[:, None])
            vone = np.zeros((S, NW), np.float32)
            vone[:, :DH] = v_t[b, rs, :].T.astype(np.float32)
            vone[:, DH] = 1.0
            m[f"qT{u}"] = qT
            m[f"kT{u}"] = kT
            m[f"kesc{u}"] = _pmajor(kesc, DH)
            m[f"vone{u}"] = _pmajor(_bf(vone), NW)
            m[f"dec{u}"] = np.ascontiguousarray(
                np.broadcast_to(g['dec'][b, h][None, :], (DH, NCH)).astype(np.float32))
            m[f"e2{u}"] = np.ascontiguousarray(
                g['e2'][b, h].reshape(NCH, LC).T.astype(np.float32))
        in_maps.append(m)
    return in_maps


def build_c_inmaps(b_results, xm_t, xg_t, wp):
    """Assemble feature-major h from phase B p-major outputs; per-core C inputs."""
    # h{u} [128p, NCH*128e] -> hT [e, s] with s = c*128+p
    hT = np.empty((B, INNER, S), dtype=ml_dtypes.bfloat16)
    for c in range(8):
        b, hp = c // 4, c % 4
        for u in range(NUNIT):
            h = 2 * hp + u
            hT[b, h * DH:(h + 1) * DH, :] = (
                b_results[c][f"h{u}"].reshape(DH, NCH, DH)
                .transpose(2, 1, 0).reshape(DH, S))
    in_maps = []
    for c in range(8):
        b, qt = c // 4, c % 4
        ts = slice(qt * TOK, (qt + 1) * TOK)
        in_maps.append(dict(
            h_i=np.ascontiguousarray(hT[b, :, ts]),
            xm_i=np.ascontiguousarray(xm_t[b, :, ts]),
            xg_i=np.ascontiguousarray(xg_t[b, :, ts]),
            skip_i=wp['skip_p'], wdT=wp['wdT']))
    return in_maps


def assemble_output(c_results):
    out = np.empty((B, S, D), np.float32)
    for c in range(8):
        b, qt = c // 4, c % 4
        out[b, qt * TOK:(qt + 1) * TOK, :] = c_results[c]['out_o'].T
    return out


# ------------------------------------------------------------------ entry ----
from concourse.bass_utils import run_bass_kernel_spmd as _run_spmd

_CACHE = {}


def _programs():
    if 'a' not in _CACHE:
        _CACHE['a'] = build_phase_a()
        _CACHE['b'] = build_phase_b()
        _CACHE['c'] = build_phase_c()
    return _CACHE['a'], _CACHE['b'], _CACHE['c']


def kernel(x, W_up, Wq, Wk, Wv, W_ig, b_ig, W_fg, b_fg, conv_w, conv_b, skip,
           W_down):
    x = np.asarray(x, np.float32)
    nc_a, nc_b, nc_c = _programs()
    cores = list(range(8))
    wp = prep_weights(W_up, Wq, Wk, Wv, W_ig, W_fg, conv_w, conv_b, skip, W_down)
    a_maps = build_a_inmaps(x, wp, b_ig, b_fg)
    ra = _run_spmd(nc_a, a_maps, core_ids=cores).results
    q_t, k_t, v_t, xm_t, xg_t, i_pre, f_pre = assemble_a_outputs(ra, b_ig, b_fg)
    g = host_gate_math(i_pre, f_pre)
    b_maps = build_b_inmaps(q_t, k_t, v_t, g)
    rb = _run_spmd(nc_b, b_maps, core_ids=cores).results
    c_maps = build_c_inmaps(rb, xm_t, xg_t, wp)
    rc = _run_spmd(nc_c, c_maps, core_ids=cores).results
    return assemble_output(rc)
